# revision 1
# baseline (speedup 1.0000x reference)
"""Trainium2 Bass kernel for Mistral-style attention with an INVERTED band mask.

Reference semantics (S=2048, E=4096, H=32, KV=8, D=128, WINDOW=1024):
  q/k/v projections -> RoPE(q,k) -> GQA attention where positions with
  |i-j| < 1024 are masked OUT (attend only to far positions) -> softmax ->
  out projection.

Sharding (8 cores, tensor-parallel by GQA group):
  core c owns KV head c and Q heads 4c..4c+3. Column-parallel QKV,
  row-parallel O projection; the 8 fp16 partial outputs are summed on host.

On-device layout: everything transposed so matmuls contract on partitions.
  Host passes hidden^T, Wq^T/Wk^T/Wv^T slices, Wo^T slice, RoPE tables
  (transposed, sign-folded), and two 128x128 triangular masks for the
  blocks that straddle the |i-j|=1024 boundary.

Block sparsity: score block (bi,bj) [128x128] is computed only when
  |bi-bj| >= 8; blocks at exactly |bi-bj|=8 get a triangular mask.

Softmax: scores^T layout [sk, sq]; exp on ScalarE (no max subtraction --
  scores are O(10) so fp32 exp is safe); denominator via ones-vector
  matmul on TensorE; normalize with reciprocal + partition_broadcast.
"""

import math
from contextlib import ExitStack

import numpy as np
import ml_dtypes

import concourse.bass as bass
import concourse.mybir as mybir
import concourse.tile as tile
from concourse import bacc
from concourse.bass_utils import run_bass_kernel_spmd

P = 128
S = 2048
E = 4096
D = 128
HPC = 4          # q heads per core
NE = E // P      # 32 e-tiles
NSCH = 4         # s-chunks of 512
SCH = S // NSCH  # 512
NST = S // P     # 16 s-tiles
NEO = 8          # output e-chunks of 512
SCALE = 1.0 / math.sqrt(D)
F16 = mybir.dt.float16
F32 = mybir.dt.float32
BF16 = mybir.dt.bfloat16


def _allowed_tiles(c):
    """For s-chunk c (query blocks bi=4c..4c+3), list (bj, lo, hi, mask, mpos):
    key tile bj is needed for query sub-tiles [lo, hi) (chunk-relative);
    mask in {None,'low','up'} applied at chunk-relative position mpos."""
    out = []
    bis = range(4 * c, 4 * c + 4)
    for bj in range(NST):
        ok = [bi for bi in bis if abs(bi - bj) >= 8]
        if not ok:
            continue
        lo = min(ok) - 4 * c
        hi = max(ok) + 1 - 4 * c
        assert ok == list(range(lo + 4 * c, hi + 4 * c)), (c, bj, ok)
        mask, mpos = None, 0
        if bj - 8 in ok:
            mask, mpos = "low", bj - 8 - 4 * c
        elif bj + 8 in ok:
            mask, mpos = "up", bj + 8 - 4 * c
        out.append((bj, lo, hi, mask, mpos))
    return out


def build_nc(debug=False):
    nc = bacc.Bacc("TRN2", target_bir_lowering=False, debug=False)
    hidT = nc.dram_tensor("hidT", (E, S), F16, kind="ExternalInput")
    wqT = nc.dram_tensor("wqT", (E, HPC * D), F16, kind="ExternalInput")
    wkT = nc.dram_tensor("wkT", (E, D), F16, kind="ExternalInput")
    wvT = nc.dram_tensor("wvT", (E, D), F16, kind="ExternalInput")
    woT = nc.dram_tensor("woT", (HPC * D, E), F16, kind="ExternalInput")
    cosT = nc.dram_tensor("cosT", (D, S), F32, kind="ExternalInput")
    sinT = nc.dram_tensor("sinT", (D, S), F32, kind="ExternalInput")
    mlow = nc.dram_tensor("mlow", (P, P), BF16, kind="ExternalInput")
    mup = nc.dram_tensor("mup", (P, P), BF16, kind="ExternalInput")
    outd = nc.dram_tensor("out", (S, E), F16, kind="ExternalOutput")
    if debug:
        qTd = nc.dram_tensor("qTd", (P, HPC, S), F16, kind="ExternalOutput")
        kTd = nc.dram_tensor("kTd", (P, S), F16, kind="ExternalOutput")
        vd = nc.dram_tensor("vd", (P, NST, D), F16, kind="ExternalOutput")
        attnd = nc.dram_tensor("attnd", (P, HPC, S), F16, kind="ExternalOutput")

    with tile.TileContext(nc) as tc, ExitStack() as ctx:
        const = ctx.enter_context(tc.tile_pool(name="const", bufs=1))

        wqT_r = wqT.rearrange("(eo p) d -> p eo d", p=P)
        wkT_r = wkT.rearrange("(eo p) d -> p eo d", p=P)
        wvT_r = wvT.rearrange("(eo p) d -> p eo d", p=P)
        wq_t, wk_t, wv_t = [], [], []
        for e in range(NE):
            wq = const.tile([P, HPC * D], F16, name=f"wq{e}")
            nc.sync.dma_start(wq[:], wqT_r[:, e, :])
            wq_t.append(wq)
            wk = const.tile([P, D], F16, name=f"wk{e}")
            nc.sync.dma_start(wk[:], wkT_r[:, e, :])
            wk_t.append(wk)
            wv = const.tile([P, D], F16, name=f"wv{e}")
            nc.sync.dma_start(wv[:], wvT_r[:, e, :])
            wv_t.append(wv)
        woT_r = woT.rearrange("(ho p) e -> p ho e", p=P)
        wo_t = []
        for h in range(HPC):
            wo = const.tile([P, E], F16, name=f"wo{h}")
            nc.sync.dma_start(wo[:], woT_r[:, h, :])
            wo_t.append(wo)
        cos_sb = const.tile([P, S], F32)
        nc.sync.dma_start(cos_sb[:], cosT[:])
        sin_sb = const.tile([P, S], F32)
        nc.sync.dma_start(sin_sb[:], sinT[:])
        ml_sb = const.tile([P, P], BF16)
        nc.sync.dma_start(ml_sb[:], mlow[:])
        mu_sb = const.tile([P, P], BF16)
        nc.sync.dma_start(mu_sb[:], mup[:])
        ones_sb = const.tile([P, 1], F16)
        nc.gpsimd.memset(ones_sb[:], 1.0)

        qT_sb = const.tile([P, HPC, S], F16)     # Q^T per head [d, s]
        kT_sb = const.tile([P, S], F16)          # K^T [d, s]
        v_sb = const.tile([P, NST, D], F16)      # V [s-tile, d]
        attn_sb = const.tile([P, HPC, S], F16)   # attn_out^T per head [d, s]

        hidp = ctx.enter_context(tc.tile_pool(name="hid", bufs=4))
        rp = ctx.enter_context(tc.tile_pool(name="rope", bufs=2))

        def rope_drain(src_psum):
            raw = rp.tile([P, SCH], F32, tag="raw", bufs=5)
            nc.any.tensor_copy(raw[:], src_psum)
            return raw

        def rope_apply(raw, dst_ap, c):
            rot = rp.tile([P, SCH], F32, tag="rot", bufs=2)
            nc.sync.dma_start(rot[0:64, :], raw[64:128, :])
            nc.sync.dma_start(rot[64:128, :], raw[0:64, :])
            t1 = rp.tile([P, SCH], F32, tag="t1", bufs=2)
            nc.vector.tensor_tensor(
                t1[:], raw[:], cos_sb[:, c * SCH:(c + 1) * SCH], mybir.AluOpType.mult)
            t2 = rp.tile([P, SCH], F32, tag="t2", bufs=2)
            nc.vector.tensor_tensor(
                t2[:], rot[:], sin_sb[:, c * SCH:(c + 1) * SCH], mybir.AluOpType.mult)
            nc.vector.tensor_tensor(dst_ap, t1[:], t2[:], mybir.AluOpType.add)

        # ---- Phase 1: QKV projections (+RoPE) ----
        with tc.tile_pool(name="p1psum", bufs=1, space="PSUM") as p1, \
             tc.tile_pool(name="p1kv", bufs=2, space="PSUM") as p1kv:
            for c in range(NSCH):
                psq = p1.tile([P, HPC, SCH], F32, tag="psq")   # 4 banks
                psk = p1kv.tile([P, SCH], F32, tag="psk")      # 2 banks
                psvT = p1kv.tile([P, SCH], F32, tag="psv")     # 2 banks (V^T)
                for e in range(NE):
                    ht = hidp.tile([P, SCH], F16, tag="hid")
                    nc.sync.dma_start(
                        ht[:], hidT[e * P:(e + 1) * P, c * SCH:(c + 1) * SCH])
                    st = (e == 0)
                    sp = (e == NE - 1)
                    for h in range(HPC):
                        nc.tensor.matmul(
                            psq[:, h, :], wq_t[e][:, h * D:(h + 1) * D], ht[:],
                            start=st, stop=sp)
                    nc.tensor.matmul(psk[:], wk_t[e][:], ht[:], start=st, stop=sp)
                    nc.tensor.matmul(psvT[:], wv_t[e][:], ht[:], start=st, stop=sp)
                vstage = rp.tile([P, SCH], F16, tag="vstage", bufs=2)
                nc.any.tensor_copy(vstage[:], psvT[:])
                nc.sync.dma_start_transpose(
                    v_sb[:, c * 4:(c + 1) * 4, :], vstage[:])
                kraw = rope_drain(psk[:])
                qraws = [rope_drain(psq[:, h, :]) for h in range(HPC)]
                rope_apply(kraw, kT_sb[:, c * SCH:(c + 1) * SCH], c)
                for h in range(HPC):
                    rope_apply(qraws[h], qT_sb[:, h, c * SCH:(c + 1) * SCH], c)

        # ---- Phase 2+3: attention interleaved with O projection ----
        ep = ctx.enter_context(tc.tile_pool(name="expp", bufs=3))
        np_pool = ctx.enter_context(tc.tile_pool(name="normp", bufs=2))
        osp = ctx.enter_context(tc.tile_pool(name="ostage", bufs=4))
        with tc.tile_pool(name="apsum", bufs=2, space="PSUM") as ap:
            for c in range(NSCH):
                blocks = _allowed_tiles(c)
                for h in range(HPC):
                    psa = ap.tile([P, SCH], F32, tag="psa")
                    psd = ap.tile([1, SCH], F32, tag="psd")
                    nblk = len(blocks)
                    for idx, (bj, lo, hi, mask, mpos) in enumerate(blocks):
                        n = (hi - lo) * P
                        pss = ap.tile([P, SCH], F32, tag="pss")
                        nc.tensor.matmul(
                            pss[:, :n],
                            kT_sb[:, bj * P:(bj + 1) * P],
                            qT_sb[:, h, c * SCH + lo * P: c * SCH + hi * P],
                            start=True, stop=True)
                        et = ep.tile([P, SCH], BF16, tag="exp")
                        if n < SCH:
                            nc.any.memzero(et[:])
                        nc.scalar.activation(
                            et[:, lo * P:hi * P], pss[:, :n],
                            mybir.ActivationFunctionType.Exp, scale=SCALE)
                        if mask == "low":
                            nc.vector.tensor_tensor(
                                et[:, mpos * P:(mpos + 1) * P],
                                et[:, mpos * P:(mpos + 1) * P],
                                ml_sb[:], mybir.AluOpType.mult)
                        elif mask == "up":
                            nc.vector.tensor_tensor(
                                et[:, mpos * P:(mpos + 1) * P],
                                et[:, mpos * P:(mpos + 1) * P],
                                mu_sb[:], mybir.AluOpType.mult)
                        nc.tensor.matmul(
                            psa[:], v_sb[:, bj, :], et[:],
                            start=(idx == 0), stop=(idx == nblk - 1))
                        nc.tensor.matmul(
                            psd[:], ones_sb[:], et[:],
                            start=(idx == 0), stop=(idx == nblk - 1))
                    rc = np_pool.tile([1, SCH], F32, tag="recip")
                    nc.vector.reciprocal(rc[:], psd[:])
                    bc = np_pool.tile([P, SCH], F32, tag="bcast")
                    nc.gpsimd.partition_broadcast(bc[:], rc[:])
                    nc.vector.tensor_tensor(
                        attn_sb[:, h, c * SCH:(c + 1) * SCH], psa[:], bc[:],
                        mybir.AluOpType.mult)
                # O projection for this chunk's four s-tiles (overlaps next
                # chunk's attention on PE via shared pool slots)
                for st in range(4 * c, 4 * c + 4):
                    orow = osp.tile([P, E], F16, tag="orow", bufs=2)
                    for eo in range(NEO):
                        pso = ap.tile([P, SCH], F32, tag="pso")
                        for h in range(HPC):
                            nc.tensor.matmul(
                                pso[:],
                                attn_sb[:, h, st * P:(st + 1) * P],
                                wo_t[h][:, eo * SCH:(eo + 1) * SCH],
                                start=(h == 0), stop=(h == HPC - 1))
                        nc.any.tensor_copy(
                            orow[:, eo * SCH:(eo + 1) * SCH], pso[:])
                    nc.sync.dma_start(outd[st * P:(st + 1) * P, :], orow[:])
        if debug:
            nc.sync.dma_start(qTd[:], qT_sb[:])
            nc.sync.dma_start(kTd[:], kT_sb[:])
            nc.sync.dma_start(vd[:], v_sb[:])
            nc.sync.dma_start(attnd[:], attn_sb[:])
    nc.compile()
    return nc


_NC_CACHE = {}


def get_nc():
    if "nc" not in _NC_CACHE:
        _NC_CACHE["nc"] = build_nc()
    return _NC_CACHE["nc"]


def make_in_maps(hidden_states, Wq, Wk, Wv, Wo):
    hid = np.asarray(hidden_states).reshape(S, E)
    hidT16 = np.ascontiguousarray(hid.T).astype(np.float16)

    inv = 1.0 / (10000.0 ** (np.arange(0, D, 2, dtype=np.float64) / D))
    t = np.arange(S, dtype=np.float64)
    fr = np.outer(t, inv)                      # [S, 64]
    emb = np.concatenate([fr, fr], axis=1)     # [S, 128]
    cosT = np.ascontiguousarray(np.cos(emb).T).astype(np.float32)
    sinT = np.ascontiguousarray(np.sin(emb).T).astype(np.float32)
    sinT[:64] *= -1.0                          # rotate_half sign fold

    jj = np.arange(P)[:, None]
    ii = np.arange(P)[None, :]
    mlow = (jj >= ii).astype(ml_dtypes.bfloat16)   # block bj-bi=8: j-i>=1024
    mup = (ii >= jj).astype(ml_dtypes.bfloat16)    # block bi-bj=8: i-j>=1024

    in_maps = []
    for c in range(8):
        qsl = slice(c * 512, (c + 1) * 512)
        ksl = slice(c * 128, (c + 1) * 128)
        in_maps.append({
            "hidT": hidT16,
            "wqT": np.ascontiguousarray(Wq[qsl].T).astype(np.float16),
            "wkT": np.ascontiguousarray(Wk[ksl].T).astype(np.float16),
            "wvT": np.ascontiguousarray(Wv[ksl].T).astype(np.float16),
            "woT": np.ascontiguousarray(Wo[:, qsl].T).astype(np.float16),
            "cosT": cosT,
            "sinT": sinT,
            "mlow": mlow,
            "mup": mup,
        })
    return in_maps


def run(in_maps, **kwargs):
    nc = get_nc()
    return run_bass_kernel_spmd(nc, in_maps, core_ids=list(range(8)), **kwargs)


def kernel(hidden_states, Wq, Wk, Wv, Wo):
    in_maps = make_in_maps(hidden_states, Wq, Wk, Wv, Wo)
    res = run(in_maps)
    out = np.zeros((S, E), dtype=np.float32)
    for r in res.results:
        out += r["out"].astype(np.float32)
    return out.reshape(1, S, E)

